# revision 9
# baseline (speedup 1.0000x reference)
"""nn_Loss_20212116095273 Trainium2 Bass kernel.

out[t,p] = 0.99 * smooth_l1(outputs - targets[...,4:8]).sum(-1)/P
           + 0.01 * 0.2/(P*15) * sum(arccos(cos_angle over 5 bbox points))

Sharding: pedestrian axis split across 8 NeuronCores (25088 padded peds/core,
196 peds per SBUF partition). Direction loss uses the identity
  arccos(cos(v1,v2)) = pi/2 - arctan(dot / |cross|)
so the per-point work is dot/cross products + Ln/Exp (for 1/|cross|) + Arctan
on the ScalarE LUT; the pi/2 * Npoints constant folds into the final bias.
Per-core partial sums of arctan are AllReduced (scalar collective), then each
core applies out = s99*raw_sl1 + (C1 - C2*gsum) and writes its map shard.
"""

import math
import numpy as np

T = 16
F = 15                     # frame pairs in direction loss
P_FULL = 200_000
N_CORES = 8
PS = 25_088                # padded peds per core (128 * 196)
J = PS // 128              # peds per partition per core = 196
JC = 28                    # peds per partition per chunk
NCH = J // JC              # 7 chunks
EPS = 1e-12

S99 = 0.99 / P_FULL
C2 = 0.01 * 0.2 / (P_FULL * F)
C1 = C2 * (P_FULL * F * 5) * (math.pi / 2.0)

_CACHE = {}


def _build(reps: int = 1):
    import concourse.bacc as bacc
    import concourse.mybir as mybir
    import concourse.tile as tile
    import concourse.bass_isa as bass_isa

    f32 = mybir.dt.float32
    bf16 = mybir.dt.bfloat16
    A = mybir.AluOpType
    AF = mybir.ActivationFunctionType

    nc = bacc.Bacc("TRN2", target_bir_lowering=False, debug=False,
                   num_devices=N_CORES)
    tgt = nc.dram_tensor("tgt", [T, PS, 8], f32, kind="ExternalInput")
    outp = nc.dram_tensor("outp", [T, PS, 4], f32, kind="ExternalInput")
    omap = nc.dram_tensor("omap", [T, PS], f32, kind="ExternalOutput")

    tgt_v = tgt.ap().rearrange("t (p j) k -> p t j k", p=128)
    out_v = outp.ap().rearrange("t (p j) k -> p t j k", p=128)
    omap_v = omap.ap().rearrange("t (p j) -> p t j", p=128)

    W = F * JC             # dir stream width per chunk (15*28 = 420)
    WS = T * JC * 4        # sl1 width (1792)
    WM = T * JC            # map width (448)

    with tile.TileContext(nc) as tc:
        with (
            tc.tile_pool(name="pin", bufs=2) as pin,
            tc.tile_pool(name="pdel", bufs=2) as pdel,
            tc.tile_pool(name="pscr", bufs=2) as pscr,
            tc.tile_pool(name="ppr", bufs=10) as ppr,
            tc.tile_pool(name="pdc", bufs=5) as pdc,
            tc.tile_pool(name="psc", bufs=6) as psc,
            tc.tile_pool(name="psd", bufs=2) as psd,
            tc.tile_pool(name="psl", bufs=6) as psl,
            tc.tile_pool(name="pmap", bufs=NCH) as pmap,
            tc.tile_pool(name="pmisc", bufs=1) as pmisc,
            tc.tile_pool(name="pdram", bufs=1, space="DRAM") as pdram,
        ):
          for _rep in range(reps):
            acc = pmisc.tile([128, NCH * 5], f32, tag="acc")
            epsb = pmisc.tile([128, 1], f32, tag="epsb")
            nc.vector.memset(epsb[:], EPS)
            raw_maps = []

            for ci in range(NCH):
                j0 = ci * JC
                tt = pin.tile([128, T * JC * 8], f32, tag="tt")
                ot = pin.tile([128, T * JC * 4], f32, tag="ot")
                nc.sync.dma_start(
                    tt[:].rearrange("p (t j k) -> p t j k", t=T, j=JC, k=8),
                    tgt_v[:, :, j0:j0 + JC, :])
                nc.sync.dma_start(
                    ot[:].rearrange("p (t j k) -> p t j k", t=T, j=JC, k=4),
                    out_v[:, :, j0:j0 + JC, :])
                t4 = tt[:].rearrange("p (t j k) -> p t j k", t=T, j=JC, k=8)
                o4 = ot[:].rearrange("p (t j k) -> p t j k", t=T, j=JC, k=4)

                def tk(r0, r1, k):
                    return t4[:, r0:r1, :, k:k + 1]

                def ok(r0, r1, k):
                    return o4[:, r0:r1, :, k:k + 1]

                # ---- stage A: 12 delta streams (x side k=(0,2), y side k=(1,3))
                deltas = {}
                for side, (ka, kc) in (("x", (0, 2)), ("y", (1, 3))):
                    a05, a1 = tk(0, F, ka), tk(1, F + 1, ka)
                    c05, c1 = tk(0, F, kc), tk(1, F + 1, kc)
                    oa, oc = ok(0, F, ka), ok(0, F, kc)
                    oa0, oc0 = ok(0, 1, ka), ok(0, 1, kc)
                    a0r, c0r = tk(0, 1, ka), tk(0, 1, kc)

                    u = pscr.tile([128, W], f32, tag="u")
                    v = pscr.tile([128, W], f32, tag="v")
                    w = pscr.tile([128, W], f32, tag="w")
                    z = pscr.tile([128, W], f32, tag="z")
                    pX0 = pdel.tile([128, W], bf16, tag="pX0" + side)
                    pX1 = pdel.tile([128, W], bf16, tag="pX1" + side)
                    tX0 = pdel.tile([128, W], bf16, tag="tX0" + side)
                    tX1 = pdel.tile([128, W], bf16, tag="tX1" + side)
                    pc = pdel.tile([128, W], bf16, tag="pc" + side)
                    tcn = pdel.tile([128, W], bf16, tag="tc" + side)
                    pX0v = pX0[:].rearrange("p (r j) -> p r j", r=F)
                    pX1v = pX1[:].rearrange("p (r j) -> p r j", r=F)
                    tX0v = tX0[:].rearrange("p (r j) -> p r j", r=F)
                    tX1v = tX1[:].rearrange("p (r j) -> p r j", r=F)
                    vv = v[:].rearrange("p (r j) -> p r j", r=F)
                    wv = w[:].rearrange("p (r j) -> p r j", r=F)
                    zv = z[:].rearrange("p (r j) -> p r j", r=F)

                    # u = 0.5*a + oa    (rows = frames 0..14)
                    nc.vector.scalar_tensor_tensor(u[:], a05, 0.5, oa, A.mult, A.add)
                    # pX0 = u - 0.5*oc ; row0 fix: oa - 0.5*oc
                    nc.vector.scalar_tensor_tensor(pX0[:], oc, -0.5, u[:], A.mult, A.add)
                    nc.vector.scalar_tensor_tensor(pX0v[:, 0:1, :], oc0, -0.5, oa0, A.mult, A.add)
                    # pX1 = 0.5*c + oa ; row0 fix: oa
                    nc.vector.scalar_tensor_tensor(pX1[:], c05, 0.5, oa, A.mult, A.add)
                    nc.scalar.copy(pX1v[:, 0:1, :], oa0)
                    nc.vector.tensor_add(pc[:], pX0[:], pX1[:])
                    # v = a(f+1) - a(f), all rows correct
                    nc.vector.tensor_sub(v[:], a1, a05)
                    # tX1 = 0.5*c + v ; row0 fix: v
                    nc.vector.scalar_tensor_tensor(tX1[:], c05, 0.5, v[:], A.mult, A.add)
                    nc.scalar.copy(tX1v[:, 0:1, :], vv[:, 0:1, :])
                    # w = a' - 0.5*c'  (frames 1..15), all rows correct
                    nc.vector.scalar_tensor_tensor(w[:], c1, -0.5, a1, A.mult, A.add)
                    # z = a - c (rows>=1) ; tX0 = w - 0.5*z
                    nc.vector.tensor_sub(z[:], a05, c05)
                    nc.vector.scalar_tensor_tensor(tX0[:], z[:], -0.5, w[:], A.mult, A.add)
                    # row0: z0 = a0 - 0.5*c0 ; tX0_0 = w0 - z0
                    nc.vector.scalar_tensor_tensor(zv[:, 0:1, :], c0r, -0.5, a0r, A.mult, A.add)
                    nc.vector.tensor_sub(tX0v[:, 0:1, :], wv[:, 0:1, :], zv[:, 0:1, :])
                    nc.vector.tensor_add(tcn[:], tX0[:], tX1[:])
                    deltas[side] = (pX0, pX1, pc, tX0, tX1, tcn)

                pX0, pX1, pcx, tX0, tX1, tcx = deltas["x"]
                pY0, pY1, pcy, tY0, tY1, tcy = deltas["y"]

                # ---- stage B: dots and crosses for the 5 points
                def mulst(a_, b_):
                    o = ppr.tile([128, W], bf16, tag="pr")
                    nc.vector.tensor_mul(o[:], a_[:], b_[:])
                    return o

                XX0 = mulst(pX0, tX0)
                XX1 = mulst(pX1, tX1)
                YY0 = mulst(pY0, tY0)
                YY1 = mulst(pY1, tY1)
                Cxx = mulst(pcx, tcx)
                Cyy = mulst(pcy, tcy)
                dots = []
                for aa, bb in ((XX0, YY0), (XX0, YY1), (XX1, YY0),
                               (XX1, YY1), (Cxx, Cyy)):
                    o = pdc.tile([128, W], bf16, tag="dot")
                    nc.vector.tensor_add(o[:], aa[:], bb[:])
                    dots.append(o)
                XY00 = mulst(pX0, tY0)
                XY01 = mulst(pX0, tY1)
                XY10 = mulst(pX1, tY0)
                XY11 = mulst(pX1, tY1)
                YX00 = mulst(pY0, tX0)
                YX10 = mulst(pY1, tX0)
                YX01 = mulst(pY0, tX1)
                YX11 = mulst(pY1, tX1)
                Cxy = mulst(pcx, tcy)
                Cyx = mulst(pcy, tcx)
                crosses = []
                for aa, bb in ((XY00, YX00), (XY01, YX10), (XY10, YX01),
                               (XY11, YX11), (Cxy, Cyx)):
                    o = pdc.tile([128, W], bf16, tag="crs")
                    nc.vector.tensor_sub(o[:], aa[:], bb[:])
                    crosses.append(o)

                # ---- stage C: theta = pi/2 - arctan(dot/|cross|)
                # 1/|cross| = exp(-0.5*ln(cross^2 + eps))
                for si in range(5):
                    c2 = psc.tile([128, W], bf16, tag="sc")
                    ln = psc.tile([128, W], bf16, tag="sc")
                    iv = psc.tile([128, W], bf16, tag="sc")
                    q = psc.tile([128, W], bf16, tag="sc")
                    at = psc.tile([128, W], bf16, tag="sc")
                    nc.vector.tensor_mul(c2[:], crosses[si][:], crosses[si][:])
                    nc.scalar.activation(ln[:], c2[:], AF.Ln, bias=epsb[:])
                    nc.scalar.activation(iv[:], ln[:], AF.Exp, scale=-0.5)
                    nc.vector.tensor_mul(q[:], dots[si][:], iv[:])
                    nc.scalar.activation(at[:], q[:], AF.Arctan,
                                         accum_out=acc[:, ci * 5 + si:ci * 5 + si + 1])

                # ---- smooth L1 map
                sd = psd.tile([128, WS], f32, tag="sd")
                nc.vector.tensor_sub(sd[:], o4, t4[:, :, :, 4:8])
                g1 = psl.tile([128, WS], bf16, tag="sl1")
                g2 = psl.tile([128, WS], bf16, tag="sl1")
                sq = psl.tile([128, WS], bf16, tag="sl1")
                msq = psl.tile([128, WS], bf16, tag="sl1")
                s1 = psl.tile([128, WS], bf16, tag="sl1")
                s2 = psl.tile([128, WS], bf16, tag="sl1")
                nc.vector.tensor_scalar(g1[:], sd[:], 1.0, 0.0, A.subtract, A.max)
                nc.vector.tensor_scalar(g2[:], sd[:], 1.0, 0.0, A.add, A.min)
                nc.vector.tensor_mul(sq[:], sd[:], sd[:])
                nc.vector.tensor_scalar(msq[:], sq[:], 1.0, None, A.min)
                nc.vector.scalar_tensor_tensor(s1[:], msq[:], 0.5, g1[:], A.mult, A.add)
                nc.vector.tensor_sub(s2[:], s1[:], g2[:])
                s2v = s2[:].rearrange("p (t j k) -> p t j k", t=T, j=JC, k=4)
                r1 = psd.tile([128, T * JC * 2], bf16, tag="r1")
                r1v = r1[:].rearrange("p (t j k) -> p t j k", t=T, j=JC, k=2)
                nc.vector.tensor_add(r1v, s2v[:, :, :, 0:2], s2v[:, :, :, 2:4])
                raw = pmap.tile([128, WM], bf16, tag="rawmap")
                rawv = raw[:].rearrange("p (t j) -> p t j", t=T)
                nc.vector.tensor_add(rawv, r1v[:, :, :, 0:1], r1v[:, :, :, 1:2])
                raw_maps.append(raw)

            # ---- global reduction of arctan partials
            accs = pmisc.tile([128, 1], f32, tag="accs")
            nc.vector.tensor_reduce(accs[:], acc[:], mybir.AxisListType.X, A.add)
            par = pmisc.tile([128, 1], f32, tag="par")
            nc.gpsimd.partition_all_reduce(par[:], accs[:], 128,
                                           bass_isa.ReduceOp.add)
            cin = pdram.tile([128, 1], f32, tag="cin")
            cout = pdram.tile([128, 1], f32, tag="cout")
            nc.sync.dma_start(cin[:], par[:])
            nc.gpsimd.collective_compute(
                "AllReduce", A.add,
                replica_groups=[list(range(N_CORES))],
                ins=[cin.opt()], outs=[cout.opt()])
            gsum = pmisc.tile([128, 1], f32, tag="gsum")
            nc.sync.dma_start(gsum[:], cout[:])
            c1t = pmisc.tile([128, 1], f32, tag="c1t")
            nc.vector.memset(c1t[:], C1)
            bias = pmisc.tile([128, 1], f32, tag="bias")
            nc.scalar.activation(bias[:], gsum[:], AF.Identity,
                                 bias=c1t[:], scale=-C2)

            # ---- finalize map shards
            for ci in range(NCH):
                j0 = ci * JC
                fin = pmap.tile([128, WM], f32, tag="fin")
                nc.scalar.activation(fin[:], raw_maps[ci][:], AF.Identity,
                                     bias=bias[:], scale=S99)
                nc.sync.dma_start(
                    omap_v[:, :, j0:j0 + JC],
                    fin[:].rearrange("p (t j) -> p t j", t=T))

    nc.compile()
    return nc


def get_program():
    if "nc" not in _CACHE:
        _CACHE["nc"] = _build()
    return _CACHE["nc"]


def kernel(outputs: np.ndarray, targets: np.ndarray) -> np.ndarray:
    from concourse import bass_utils

    nc = get_program()
    outputs = np.ascontiguousarray(outputs, dtype=np.float32)
    targets = np.ascontiguousarray(targets, dtype=np.float32)

    in_maps = []
    for c in range(N_CORES):
        lo = c * 25000
        t_s = np.zeros((T, PS, 8), np.float32)
        o_s = np.zeros((T, PS, 4), np.float32)
        t_s[:, :25000] = targets[:, lo:lo + 25000]
        o_s[:, :25000] = outputs[:, lo:lo + 25000]
        in_maps.append({"tgt": t_s, "outp": o_s})

    res = bass_utils.run_bass_kernel_spmd(nc, in_maps,
                                          core_ids=list(range(N_CORES)))
    out = np.empty((T, P_FULL), np.float32)
    for c in range(N_CORES):
        out[:, c * 25000:(c + 1) * 25000] = res.results[c]["omap"][:, :25000]
    return out
